# revision 21
# baseline (speedup 1.0000x reference)
"""CornerPooling fused kernel for 8 Trainium2 NeuronCores — Winograd F(2,3).

Network (per sample):
  p1 = TopPool(relu(bn(conv3x3(x, w_p1))))      # reverse cummax along H
  p2 = LeftPool(relu(bn(conv3x3(x, w_p2))))     # reverse cummax along W
  t  = bn(conv3x3(p1 + p2, w_pool))
  u  = bn(conv1x1(x, w_c1))
  out = relu(bn(conv3x3(relu(t + u), w_c2)))

Strategy: data-parallel, one sample per core. All 3x3 convs use 1-D Winograd
F(2,3) along W (ky stays a direct 3-term sum): per output row, 4 transform
points of 64 tiles replace 3 kx-taps of 128 cols -> 1.5x fewer PE columns.
  U1 = d0-d2, U2 = d1+d2, U3 = d2-d1, U4 = d1-d3      (DVE, strided reads)
  m_p = sum_ky  G_p[ky]  @ U_p[row+ky]                (PE, PSUM accumulate)
  y0 = m1+m2+m3, y1 = m2-m3-m4                        (scalar evict + DVE)
The scalar engine evicts m-planes PSUM->SBUF bf16 (folding bias into m1 and
bias/negation into m4), DVE combines them into y0/y1. conv1x1(x) accumulates
into pool's m1 (even cols) / m4 with negated weights (odd cols), so A^T
absorbs it for free. ReLU before the pools is folded into the cummax scans
(init 0 / masked restart at 0); ReLU after pool/c2 is a 4x-mode DVE max.

The whole net runs as ONE bottom-up row pipeline (group g covers rows
[8g, 8g+8), g = 15..0): p1p2(g) -> scans(g) -> pool(g+2) -> c2(g+3), so the
reverse-cummax along H is an 8-row chained row-max and every engine's work is
spread evenly under the PE stream. a1/r/y/x live in SBUF row-rings.
"""

import numpy as np
import ml_dtypes

import concourse.bass as bass
import concourse.mybir as mybir
import concourse.tile as tile
from concourse import bacc
from concourse.bass_utils import run_bass_kernel_spmd

BF16 = mybir.dt.bfloat16
F32 = mybir.dt.float32
NPBF16 = ml_dtypes.bfloat16

DEBUG = False
ISOLATE = ""
NO_C1 = False
N_CORES = 8
C, MID = 256, 128
H = W = 128
HP, WP = H + 2, W + 2
J = W // 2            # 64 winograd tiles per row
NG = H // 8           # 16 8-row macro groups
XR = 48               # x ring rows (padded row index % XR)
AR = 32               # a1 ring rows
RR = 32               # r ring rows (padded row index % RR)
YR = 16               # y ring rows


def _segs(r0, n, R):
    """Split ring-row range [r0, r0+n) into contiguous (off, slot, cnt)."""
    out = []
    off = 0
    while n > 0:
        s = (r0 + off) % R
        c = min(n, R - s)
        out.append((off, s, c))
        off += c
        n -= c
    return out


def _emit(tc, d):
    nc = tc.nc
    AOp = mybir.AluOpType
    Ident = mybir.ActivationFunctionType.Identity
    Copy = mybir.ActivationFunctionType.Copy

    from contextlib import ExitStack
    ctx = ExitStack()
    const = ctx.enter_context(tc.tile_pool(name="const", bufs=1))
    ring = ctx.enter_context(tc.tile_pool(name="ring", bufs=1))
    uxp = ctx.enter_context(tc.tile_pool(name="uxp", bufs=2))
    usp = ctx.enter_context(tc.tile_pool(name="usp", bufs=2))
    urp = ctx.enter_context(tc.tile_pool(name="urp", bufs=2))
    stp = ctx.enter_context(tc.tile_pool(name="stp", bufs=5))
    tbp = ctx.enter_context(tc.tile_pool(name="tbp", bufs=4))
    psum = ctx.enter_context(tc.tile_pool(name="psum", bufs=4, space="PSUM"))

    # ---- weights / bias (sync ring; w1 first gates the first matmuls) ----
    biassb = const.tile([128, 6], F32, tag="bias", name="bias_sb")
    w1sb = [const.tile([128, 12 * MID], BF16, tag=f"w1{k}", name=f"w1_{k}") for k in range(2)]
    w2sb = [const.tile([128, 12 * MID], BF16, tag=f"w2{k}", name=f"w2_{k}") for k in range(2)]
    wpsb = const.tile([128, 12 * C], BF16, tag="wp", name="wp_sb")
    wc1sb = [const.tile([128, 2 * C], BF16, tag=f"wc1{k}", name=f"wc1_{k}") for k in range(2)]
    wc2sb = [const.tile([128, 12 * C], BF16, tag=f"wc2{k}", name=f"wc2_{k}") for k in range(2)]
    nc.sync.dma_start(biassb[:], d["bias"][:])
    for k in range(2):
        nc.sync.dma_start(w1sb[k][:], d["w1"][k])
    for k in range(2):
        nc.sync.dma_start(w2sb[k][:], d["w2"][k])
    nc.sync.dma_start(wpsb[:], d["wp"][:])
    for k in range(2):
        nc.sync.dma_start(wc1sb[k][:], d["wc1"][k])
    for k in range(2):
        nc.sync.dma_start(wc2sb[k][:], d["wc2"][k])

    # ---- PE p-state warmup off a memset tile while DMAs fly ----
    warm = const.tile([128, 640], BF16, tag="warm", name="warm")
    nc.gpsimd.memset(warm[:], 1.0)
    wps = psum.tile([128, 4, 256], F32, tag="ps", name="warm_ps")
    for i in range(48):
        nc.tensor.matmul(wps[:, 0, :], warm[:, 0:128], warm[:, 128:384],
                         start=(i == 0), stop=(i == 47))

    # ---- rings / buffers ----
    xr = [ring.tile([128, XR, WP], BF16, tag=f"xr{k}", name=f"xr_{k}") for k in range(2)]
    a1r = ring.tile([128, 2, AR, J], BF16, tag="a1r", name="a1r")
    a2f = ring.tile([128, HP, WP], BF16, tag="a2f", name="a2f")
    rr_ = [ring.tile([128, 2, RR, 66], BF16, tag=f"rr{m}", name=f"rr_{m}") for m in range(2)]
    yr_ = [ring.tile([128, YR, 2, J], BF16, tag=f"yr{m}", name=f"yr_{m}") for m in range(2)]
    maskl = const.tile([128, 8 * WP], BF16, tag="maskl", name="maskl")

    # pads: a2 row 0 / 129 and col 0 / 129 must be zero; r rings fully zero
    nc.vector.memset(a2f[:, 0, :], 0.0)
    nc.vector.memset(a2f[:, HP - 1, :], 0.0)
    nc.vector.memset(a2f[:, :, 0], 0.0)
    nc.vector.memset(a2f[:, :, WP - 1], 0.0)
    for m in range(2):
        nc.vector.memset(rr_[m][:], 0.0)
    nc.vector.memset(maskl[:], 1.0)
    mlv = maskl[:].rearrange("p (r c) -> p r c", c=WP)
    nc.vector.memset(mlv[:, :, 0], 0.0)
    nc.vector.memset(mlv[:, :, WP - 1], 0.0)

    xpd = [d["xp"][k].rearrange("p (h w) -> p h w", w=WP) for k in range(2)]
    yv = [d["y"][m].rearrange("p (h a j) -> p h a j", a=2, j=J) for m in range(2)]

    # ---------------- helpers ----------------
    def u_points_x(kb):
        xv = xr[kb]
        def f(p, sl, c):
            rows = xv[:, sl:sl + c, :]
            if p == 0:
                return rows[:, :, 0:W:2], rows[:, :, 2:WP:2], AOp.subtract
            if p == 1:
                return rows[:, :, 1:W + 1:2], rows[:, :, 2:WP:2], AOp.add
            if p == 2:
                return rows[:, :, 2:WP:2], rows[:, :, 1:W + 1:2], AOp.subtract
            return rows[:, :, 1:W + 1:2], rows[:, :, 3:WP:2], AOp.subtract
        return f

    def u_points_s(p, sl, c):
        rows = a2f[:, sl:sl + c, :]
        if p == 0:
            return rows[:, :, 0:W:2], rows[:, :, 2:WP:2], AOp.subtract
        if p == 1:
            return rows[:, :, 1:W + 1:2], rows[:, :, 2:WP:2], AOp.add
        if p == 2:
            return rows[:, :, 2:WP:2], rows[:, :, 1:W + 1:2], AOp.subtract
        return rows[:, :, 1:W + 1:2], rows[:, :, 3:WP:2], AOp.subtract

    def u_points_r(kb):
        rv = rr_[kb]
        def f(p, sl, c):
            O = rv[:, 0, sl:sl + c, :]
            E = rv[:, 1, sl:sl + c, :]
            if p == 0:
                return E[:, :, 0:J], E[:, :, 1:J + 1], AOp.subtract
            if p == 1:
                return O[:, :, 0:J], E[:, :, 1:J + 1], AOp.add
            if p == 2:
                return E[:, :, 1:J + 1], O[:, :, 0:J], AOp.subtract
            return O[:, :, 0:J], O[:, :, 1:J + 1], AOp.subtract
        return f

    def emit_u(dst, fin, r0, n, R, row_lo=0):
        """dst [128, n, 4, J] <- U points of ring rows [r0+row_lo, r0+n)."""
        for off, sl, c in _segs(r0 + row_lo, n - row_lo, R):
            o = off + row_lo
            for p in range(4):
                i0, i1, op = fin(p, sl, c)
                nc.vector.tensor_tensor(dst[:, o:o + c, p, :], i0, i1, op)

    def wino_mms(psv, uv, wtiles, O, mb, c1r0=None):
        """psv [128,4,4,J]: accumulate the 12 winograd taps, PLANE-MAJOR.

        A group's start=True clears has_written for the WHOLE bank, so the
        4 planes' accumulation chains must not interleave within a bank
        (data of finished planes survives later clears; bits don't matter).
        With c1r0 set (pool), conv1x1(x) rides plane 0 (even cols, +w) and
        plane 3 (odd cols, -w) before those planes close."""
        KB = len(wtiles)
        for p in range(4):
            for kb in range(KB):
                for ky in range(3):
                    col = (ky * 4 + p) * O + mb * 128
                    stop = (kb == KB - 1 and ky == 2) and not (
                        c1r0 is not None and p in (0, 3) and not NO_C1)
                    nc.tensor.matmul(psv[:, p], wtiles[kb][:, col:col + 128],
                                     uv[kb][:, ky:ky + 4, p, :],
                                     start=(kb == 0 and ky == 0), stop=stop)
            if c1r0 is not None and p in (0, 3) and not NO_C1:
                ss = _segs(c1r0 + 1, 4, XR)
                for kb in range(2):
                    for si, (off, sl, c) in enumerate(ss):
                        rows = xr[kb][:, sl:sl + c, :]
                        last = kb == 1 and si == len(ss) - 1
                        wcol = (0 if p == 0 else C) + mb * 128
                        rhs = rows[:, :, 1:W + 1:2] if p == 0 else rows[:, :, 2:WP:2]
                        nc.tensor.matmul(psv[:, p, off:off + c, :],
                                         wc1sb[kb][:, wcol:wcol + 128],
                                         rhs, start=False, stop=last)

    def evict(ps, st, bcol):
        """PSUM m-planes -> SBUF bf16: [m2|m3|m1+b|-m4+b]."""
        nc.scalar.activation(st[:, 0:2, :], ps[:, 1:3, :], Copy)
        nc.scalar.activation(st[:, 2, :], ps[:, 0, :], Ident,
                             bias=biassb[:, bcol:bcol + 1])
        nc.scalar.activation(st[:, 3, :], ps[:, 3, :], Ident,
                             bias=biassb[:, bcol:bcol + 1], scale=-1.0)

    def comb_t(st, tb):
        nc.vector.tensor_add(tb[:, 0, :], st[:, 0, :], st[:, 1, :])
        nc.vector.tensor_sub(tb[:, 1, :], st[:, 0, :], st[:, 1, :])

    def p_group(g, half, conv):  # conv: 0 = p1, 1 = p2
        r0 = 8 * g + 4 * half
        uo = 4 * half
        ps = psum.tile([128, 4, 1024 // 4], F32, tag="ps", name="ps")
        psv = ps[:].rearrange("p q (r j) -> p q r j", j=J)
        uv = [uxt[k][:, uo:uo + 6] for k in range(2)]
        wino_mms(psv, uv, w1sb if conv == 0 else w2sb, MID, 0)
        st = stp.tile([128, 4, 256], BF16, tag="st", name="st")
        evict(ps[:], st[:], conv)
        tb = tbp.tile([128, 2, 256], BF16, tag="tb", name="tb")
        comb_t(st[:], tb[:])
        tv = tb[:].rearrange("p a (r j) -> p a r j", j=J)
        sv = st[:, 2:4, :].rearrange("p a (r j) -> p a r j", j=J)
        if conv == 0:
            out = a1r[:, :, (r0 % AR):(r0 % AR) + 4, :]
        else:
            out = a2f[:, r0 + 1:r0 + 5, 1:W + 1].rearrange(
                "p r (j t) -> p t r j", t=2)
        nc.vector.tensor_add(out, tv, sv)

    def pool_group(gp, half):
        r0 = 8 * gp + 4 * half
        uo = 4 * half
        for mb in range(2):
            ps = psum.tile([128, 4, 256], F32, tag="ps", name="ps")
            psv = ps[:].rearrange("p q (r j) -> p q r j", j=J)
            wino_mms(psv, [ust[:, uo:uo + 6]], [wpsb], C, mb, c1r0=r0)
            st = stp.tile([128, 4, 256], BF16, tag="st", name="st")
            evict(ps[:], st[:], 2 + mb)
            tb = tbp.tile([128, 2, 256], BF16, tag="tb", name="tb")
            comb_t(st[:], tb[:])
            tv = tb[:].rearrange("p a (r j) -> p a r j", j=J)
            sv = st[:, 2:4, :].rearrange("p a (r j) -> p a r j", j=J)
            for off, sl, c in _segs(r0 + 1, 4, RR):
                nc.vector.tensor_add(rr_[mb][:, 0, sl:sl + c, 0:J],
                                     tv[:, 0, off:off + c, :],
                                     sv[:, 0, off:off + c, :])
                nc.vector.tensor_add(rr_[mb][:, 1, sl:sl + c, 1:J + 1],
                                     tv[:, 1, off:off + c, :],
                                     sv[:, 1, off:off + c, :])
                nc.vector.tensor_scalar_max(rr_[mb][:, :, sl:sl + c, :],
                                            rr_[mb][:, :, sl:sl + c, :], 0.0)

    def c2_group(gc, half):
        r0 = 8 * gc + 4 * half
        uo = 4 * half
        ys = r0 % YR
        for mb in range(2):
            ps = psum.tile([128, 4, 256], F32, tag="ps", name="ps")
            psv = ps[:].rearrange("p q (r j) -> p q r j", j=J)
            wino_mms(psv, [urt[k][:, uo:uo + 6] for k in range(2)], wc2sb, C, mb)
            st = stp.tile([128, 4, 256], BF16, tag="st", name="st")
            evict(ps[:], st[:], 4 + mb)
            tb = tbp.tile([128, 2, 256], BF16, tag="tb", name="tb")
            comb_t(st[:], tb[:])
            tv = tb[:].rearrange("p a (r j) -> p a r j", j=J)
            sv = st[:, 2:4, :].rearrange("p a (r j) -> p a r j", j=J)
            blk = yr_[mb][:, ys:ys + 4, :, :]
            out = blk.rearrange("p r a j -> p a r j")
            nc.vector.tensor_add(out, tv, sv)
            nc.vector.tensor_scalar_max(blk, blk, 0.0)
            nc.sync.dma_start(
                yv[mb][:, r0:r0 + 4, :, :].rearrange("p r a j -> p (r a j)"),
                blk.rearrange("p r a j -> p (r a j)"))

    def dma_x_chunk(cN):
        if cN == -1:
            lo, n = 0, 2
        else:
            lo, n = 8 * cN + 2, 8
        for off, sl, c in _segs(lo, n, XR):
            for k in range(2):
                nc.scalar.dma_start(xr[k][:, sl:sl + c, :],
                                    xpd[k][:, lo + off:lo + off + c, :])

    # ---- bootstrap: x chunks 15, 14 then Ux(15) ----
    dma_x_chunk(NG - 1)
    dma_x_chunk(NG - 2)
    uxt = [uxp.tile([128, 10, 4, J], BF16, tag=f"ux{k}", name=f"ux_{k}")
           for k in range(2)]
    for k in range(2):
        emit_u(uxt[k], u_points_x(k), 8 * (NG - 1), 10, XR)

    ust = None
    urt = None

    # ---------------- main pipeline ----------------
    for i in range(NG + 4):
        g = NG - 1 - i
        if -1 <= g - 2:
            dma_x_chunk(g - 2)

        gc = g + 4
        if 0 <= gc <= NG - 1:
            # Ur rows [8gc, 8gc+10): r written >= 2 iterations ago
            urt = [urp.tile([128, 10, 4, J], BF16, tag=f"ur{k}", name=f"ur_{k}")
                   for k in range(2)]
            for k in range(2):
                emit_u(urt[k], u_points_r(k), 8 * gc, 10, RR)

        if 0 <= g:
            for half in range(2):
                p_group(g, half, 0)
            if not ISOLATE:
                for half in range(2):
                    p_group(g, half, 1)
            if DEBUG:
                da1v = d["da1"].rearrange("p (a h j) -> p a h j", h=H, j=J)
                nc.sync.dma_start(
                    da1v[:, :, 8 * g:8 * g + 8, :],
                    a1r[:, :, (8 * g) % AR:(8 * g) % AR + 8, :])
            # top-pool: chained row-max (relu via max(127,0) seed)
            if ISOLATE:
                if 1 <= g:
                    uxt = [uxp.tile([128, 10, 4, J], BF16, tag=f"ux{k}", name=f"ux_{k}")
                           for k in range(2)]
                    for k in range(2):
                        emit_u(uxt[k], u_points_x(k), 8 * (g - 1), 10, XR)
                continue
            hi = 8 * g + 7
            if g == NG - 1:
                nc.vector.tensor_scalar_max(a1r[:, :, hi % AR, :],
                                            a1r[:, :, hi % AR, :], 0.0)
                hs = hi - 1
            else:
                hs = hi
            for h in range(hs, 8 * g - 1, -1):
                nc.vector.tensor_max(a1r[:, :, h % AR, :],
                                     a1r[:, :, h % AR, :],
                                     a1r[:, :, (h + 1) % AR, :])
            # left-pool: masked reverse scan over 8 padded rows
            fl = a2f[:].rearrange("p h w -> p (h w)")
            chunk = fl[:, (8 * g + 1) * WP:(8 * g + 9) * WP]
            nc.vector.tensor_tensor_scan(chunk[:, ::-1], maskl[:],
                                         chunk[:, ::-1], 0.0,
                                         op0=AOp.mult, op1=AOp.max)
            # s = p1 + p2 over these 8 rows
            sa = a1r[:, :, (8 * g) % AR:(8 * g) % AR + 8, :]
            nc.vector.tensor_add(
                a2f[:, 8 * g + 1:8 * g + 9, 1:W + 1:2],
                a2f[:, 8 * g + 1:8 * g + 9, 1:W + 1:2], sa[:, 0])
            nc.vector.tensor_add(
                a2f[:, 8 * g + 1:8 * g + 9, 2:WP:2],
                a2f[:, 8 * g + 1:8 * g + 9, 2:WP:2], sa[:, 1])

        gp = g + 2
        if 0 <= gp <= NG - 1:
            for half in range(2):
                pool_group(gp, half)

        # Us for NEXT iteration's pool group — after this iteration's pool
        # has consumed the previous chunk (ust rebind must follow its use)
        gs = g + 1
        if 0 <= gs <= NG - 1:
            ust = usp.tile([128, 10, 4, J], BF16, tag="us", name="us")
            emit_u(ust, u_points_s, 8 * gs, 10, 10 ** 9)
            if DEBUG:
                for m in range(2):
                    drv = d["dr"][m].rearrange("p (a h c) -> p a h c", h=HP, c=66)
                    for off, sl, cc in _segs(8 * gp + 1, 8, RR):
                        nc.sync.dma_start(
                            drv[:, :, 8 * gp + 1 + off:8 * gp + 1 + off + cc, :],
                            rr_[m][:, :, sl:sl + cc, :])

        if 0 <= gc <= NG - 1:
            if gc == 0:
                c2_group(gc, 0)
                for r0 in (4, 6):   # 2-row tail pieces: short final drain
                    ys = r0 % YR
                    for mb in range(2):
                        ps = psum.tile([128, 4, 256], F32, tag="ps", name="ps")
                        psv = ps[:].rearrange("p q (r j) -> p q r j", j=J)
                        for p in range(4):
                            for kb in range(2):
                                for ky in range(3):
                                    col = (ky * 4 + p) * C + mb * 128
                                    nc.tensor.matmul(
                                        psv[:, p, 0:2, :],
                                        wc2sb[kb][:, col:col + 128],
                                        urt[kb][:, r0 + ky:r0 + ky + 2, p, :],
                                        start=(kb == 0 and ky == 0),
                                        stop=(kb == 1 and ky == 2))
                        st = stp.tile([128, 4, 256], BF16, tag="st", name="st")
                        nc.scalar.activation(st[:, :, 0:128], ps[:, :, 0:128], Copy)
                        tb = tbp.tile([128, 2, 256], BF16, tag="tb", name="tb")
                        nc.vector.tensor_add(tb[:, 0, 0:128], st[:, 0, 0:128], st[:, 1, 0:128])
                        nc.vector.tensor_sub(tb[:, 1, 0:128], st[:, 0, 0:128], st[:, 1, 0:128])
                        tv = tb[:, :, 0:128].rearrange("p a (r j) -> p a r j", j=J)
                        sv = st[:, 2:4, 0:128].rearrange("p a (r j) -> p a r j", j=J)
                        blk = yr_[mb][:, ys:ys + 2, :, :]
                        nc.vector.tensor_add(blk.rearrange("p r a j -> p a r j"), tv, sv)
                        nc.vector.tensor_scalar(blk, blk, biassb[:, 4 + mb:5 + mb],
                                                0.0, op0=AOp.add, op1=AOp.max)
                        nc.sync.dma_start(
                            yv[mb][:, r0:r0 + 2, :, :].rearrange("p r a j -> p (r a j)"),
                            blk.rearrange("p r a j -> p (r a j)"))
            else:
                for half in range(2):
                    c2_group(gc, half)

        if 1 <= g:
            uxt = [uxp.tile([128, 10, 4, J], BF16, tag=f"ux{k}", name=f"ux_{k}")
                   for k in range(2)]
            for k in range(2):
                emit_u(uxt[k], u_points_x(k), 8 * (g - 1), 10, XR)

        if i == NG:
            # slot 0 of the r rings flips from "padded row 128" to the
            # top pad (padded row 0) for the last c2 group
            for m in range(2):
                nc.vector.memset(rr_[m][:, :, 0, :], 0.0)

    if DEBUG:
        nc.sync.dma_start(d["da2"][:], a2f[:].rearrange("p h w -> p (h w)"))

    ctx.close()


_MODULE_CACHE = {}


def build_module(reps=1):
    key = reps
    if key in _MODULE_CACHE:
        return _MODULE_CACHE[key]
    SPP = HP * WP
    nc = bacc.Bacc("TRN2", debug=False)
    d = {}
    d["xp"] = nc.dram_tensor("xp", [2, 128, SPP], BF16, kind="ExternalInput").ap()
    d["w1"] = nc.dram_tensor("w1", [2, 128, 12 * MID], BF16, kind="ExternalInput").ap()
    d["w2"] = nc.dram_tensor("w2", [2, 128, 12 * MID], BF16, kind="ExternalInput").ap()
    d["wp"] = nc.dram_tensor("wp", [128, 12 * C], BF16, kind="ExternalInput").ap()
    d["wc1"] = nc.dram_tensor("wc1", [2, 128, 2 * C], BF16, kind="ExternalInput").ap()
    d["wc2"] = nc.dram_tensor("wc2", [2, 128, 12 * C], BF16, kind="ExternalInput").ap()
    d["bias"] = nc.dram_tensor("bias", [128, 6], F32, kind="ExternalInput").ap()
    d["y"] = nc.dram_tensor("y", [2, 128, 2 * H * J], BF16, kind="ExternalOutput").ap()
    if DEBUG:
        d["da1"] = nc.dram_tensor("da1", [128, 2 * H * J], BF16, kind="ExternalOutput").ap()
        d["da2"] = nc.dram_tensor("da2", [128, HP * WP], BF16, kind="ExternalOutput").ap()
        d["dr"] = nc.dram_tensor("dr", [2, 128, 2 * HP * 66], BF16, kind="ExternalOutput").ap()
    with tile.TileContext(nc) as tc:
        for _ in range(reps):
            _emit(tc, d)
    nc.compile()
    _MODULE_CACHE[key] = nc
    return nc


def _fold(w, g, b, m, v, eps=1e-5):
    inv = g / np.sqrt(v + eps)
    return (w * inv[:, None, None, None]).astype(np.float32), (b - m * inv).astype(np.float32)


def _wino_w(w):
    """[O, I, 3, 3] -> [I//128, 128, 12*O] bf16, col = (ky*4+p)*O + o."""
    O, I = w.shape[0], w.shape[1]
    g0, g1, g2 = w[..., 0], w[..., 1], w[..., 2]          # [O, I, ky]
    pts = np.stack([g0, (g0 + g1 + g2) / 2, (g0 - g1 + g2) / 2, g2], 0)
    t = np.transpose(pts, (2, 3, 0, 1))                   # [I, ky, 4, O]
    t = np.ascontiguousarray(t).reshape(I, 12 * O)
    return t.reshape(I // 128, 128, 12 * O).astype(NPBF16)


def prep_host(inputs):
    """Fold BN, winograd-transform weights, pad+cast x."""
    w1f, b1 = _fold(inputs["w_p1"], inputs["g_p1"], inputs["b_p1"], inputs["m_p1"], inputs["v_p1"])
    w2f, b2 = _fold(inputs["w_p2"], inputs["g_p2"], inputs["b_p2"], inputs["m_p2"], inputs["v_p2"])
    wpf, bp = _fold(inputs["w_pool"], inputs["g_pool"], inputs["b_pool"], inputs["m_pool"], inputs["v_pool"])
    wc1f, bc1 = _fold(inputs["w_c1"], inputs["g_c1"], inputs["b_c1"], inputs["m_c1"], inputs["v_c1"])
    wc2f, bc2 = _fold(inputs["w_c2"], inputs["g_c2"], inputs["b_c2"], inputs["m_c2"], inputs["v_c2"])
    br = bp + bc1

    bias = np.zeros((128, 6), np.float32)
    bias[:, 0] = b1
    bias[:, 1] = b2
    bias[:, 2] = br[:128]
    bias[:, 3] = br[128:]
    bias[:, 4] = bc2[:128]
    bias[:, 5] = bc2[128:]

    # c1: [O, I] -> per cin-block [128, 2*C]: cols [0,C) = +w (even),
    # [C,2C) = -w (odd)
    wc1m = wc1f[:, :, 0, 0]                                # [O, I]
    wc1t = np.ascontiguousarray(wc1m.T)                    # [I, O]
    wc1pk = np.concatenate([wc1t, -wc1t], axis=1)          # [I, 2C]
    wc1pk = wc1pk.reshape(2, 128, 2 * C).astype(NPBF16)

    shared = {
        "w1": _wino_w(w1f), "w2": _wino_w(w2f),
        "wp": _wino_w(wpf)[0], "wc1": wc1pk, "wc2": _wino_w(wc2f),
        "bias": bias,
    }

    x = np.asarray(inputs["in_feature"], np.float32)       # [N, 256, H, W]
    N = x.shape[0]
    xp = np.zeros((N, 2, 128, HP, WP), NPBF16)
    xp[:, :, :, 1:1 + H, 1:1 + W] = x.reshape(N, 2, 128, H, W).astype(NPBF16)
    xp = xp.reshape(N, 2, 128, HP * WP)
    return shared, xp


def kernel(**inputs):
    nc = build_module()
    shared, xp = prep_host(inputs)
    n = xp.shape[0]
    in_maps = [dict(shared, xp=np.ascontiguousarray(xp[i])) for i in range(n)]
    res = run_bass_kernel_spmd(nc, in_maps, core_ids=list(range(n)))
    outs = []
    for r in res.results:
        y = np.asarray(r["y"], np.float32).reshape(2, 128, H, 2, J)
        full = np.empty((2, 128, H, W), np.float32)
        full[..., 0::2] = y[:, :, :, 0]
        full[..., 1::2] = y[:, :, :, 1]
        outs.append(full.reshape(C, H, W))
    return np.stack(outs).astype(np.float32)
